# revision 1
# baseline (speedup 1.0000x reference)
"""BinaryLinear kernel for 8 Trainium2 NeuronCores.

y = x @ (scale * sign(weight))^T,  x:[8192,4096] f32, weight:[4096,4096] f32.

Strategy: data-parallel token split (1024 tokens/core), weight replicated.
Per core: x*scale cast to fp16 (resident in SBUF, [K,T] layout), weight
streamed in [128,512] f32 chunks and binarized to +/-1 fp16 on ScalarE
(Sign), fp16 matmuls (K=128 contraction tiles) accumulate f32 in PSUM,
VectorE drains PSUM->SBUF, gpsimd DMA stores out (separate ring so pending
stores never block weight prefetch on the sync HWDGE ring).

Loop order is k-outer with all 8 token-tiles accumulating in lockstep
across the 8 PSUM banks, so the PE consumes each (x,w) chunk pair as it
arrives during the initial load window. The first weight slab's DMAs are
interleaved with the x loads on the sync ring (FIFO per ring) so the PE
starts within a few microseconds.
"""

import numpy as np

TOKENS = 8192
IN_F = 4096
OUT_F = 4096
N_CORES = 8
TS = TOKENS // N_CORES  # tokens per core

P = 128        # partitions / contraction tile
N_TILE = 512   # matmul moving free dim (one PSUM bank of f32)
K_TILES = IN_F // P          # 32
T_TILES = TS // P            # 8
O_TILES = OUT_F // N_TILE    # 8
PSUM_BUFS = 8


def _build_program(scale: float):
    import concourse.bacc as bacc
    import concourse.mybir as mybir
    import concourse.tile as tile

    fp32 = mybir.dt.float32
    fp16 = mybir.dt.float16

    nc = bacc.Bacc(
        "TRN2",
        target_bir_lowering=False,
        debug=False,
        num_devices=N_CORES,
    )
    xt_d = nc.dram_tensor("xt", [IN_F, TS], fp32, kind="ExternalInput").ap()
    wt_d = nc.dram_tensor("wt", [IN_F, OUT_F], fp32, kind="ExternalInput").ap()
    y_d = nc.dram_tensor("y", [TS, OUT_F], fp32, kind="ExternalOutput").ap()

    scratch_d = nc.dram_tensor("scratch", [P, N_TILE], fp32, kind="Internal").ap()

    with tile.TileContext(nc) as tc:
        with (
            tc.tile_pool(name="xres", bufs=K_TILES) as xres_pool,
            tc.tile_pool(name="wchunk", bufs=48) as wchunk_pool,
            tc.tile_pool(name="xstage", bufs=8) as xstage_pool,
            tc.tile_pool(name="wstage", bufs=12) as wstage_pool,
            tc.tile_pool(name="ostage", bufs=8) as ostage_pool,
            tc.tile_pool(name="warm", bufs=1) as warm_pool,
            tc.tile_pool(name="psum", bufs=PSUM_BUFS, space="PSUM") as psum_pool,
        ):
            # Warm-up at t=0 (no data deps): preload the ACT Sign LUT and
            # run dummy matmuls so the PE HAM clock-gate reaches 2.4 GHz
            # before the first real matmul. Chain ends in a store to an
            # internal scratch tensor so nothing here is dead code.
            warm_f = warm_pool.tile([P, N_TILE], fp32)
            nc.gpsimd.memset(warm_f[:], 0.0)
            warm_h = warm_pool.tile([P, N_TILE], fp16)
            nc.scalar.sign(warm_h[:], warm_f[:])
            warm_ps = psum_pool.tile([P, N_TILE], fp32, tag="ps", name="warm_ps")
            N_WARM = 40
            for i in range(N_WARM):
                nc.tensor.matmul(
                    warm_ps[:],
                    warm_h[:, 0:P],
                    warm_h[:],
                    start=(i == 0),
                    stop=(i == N_WARM - 1),
                )
            warm_o = warm_pool.tile([P, N_TILE], fp32)
            nc.vector.tensor_copy(warm_o[:], warm_ps[:])
            nc.gpsimd.dma_start(scratch_d[:], warm_o[:])

            xs = []   # resident fp16 x^T chunks, [P, TS] each
            wb0 = []  # first slab's binarized chunks

            def load_w_chunk(o, k):
                wf = wstage_pool.tile([P, N_TILE], fp32, tag="wf")
                nc.sync.dma_start(
                    wf[:],
                    wt_d[k * P : (k + 1) * P, o * N_TILE : (o + 1) * N_TILE],
                )
                wc = wchunk_pool.tile([P, N_TILE], fp16, tag="wc", name="wc")
                nc.scalar.sign(wc[:], wf[:])
                return wc

            # Phase A: interleave x chunk loads with the first w slab's
            # chunks so the PE can start as soon as pair 0 lands. The first
            # x chunk is split so the first matmul only waits on 64 KB.
            for k in range(K_TILES):
                if k == 0:
                    wb0.append(load_w_chunk(0, 0))
                xf = xstage_pool.tile([P, TS], fp32, tag="xf")
                xk = xres_pool.tile([P, TS], fp16, tag="xs")
                if k == 0:
                    nc.sync.dma_start(xf[:, 0:P], xt_d[0:P, 0:P])
                    nc.vector.tensor_scalar_mul(xk[:, 0:P], xf[:, 0:P], float(scale))
                    nc.sync.dma_start(xf[:, P:TS], xt_d[0:P, P:TS])
                    nc.vector.tensor_scalar_mul(xk[:, P:TS], xf[:, P:TS], float(scale))
                else:
                    nc.sync.dma_start(xf[:], xt_d[k * P : (k + 1) * P, :])
                    nc.vector.tensor_scalar_mul(xk[:], xf[:], float(scale))
                xs.append(xk)
                if k > 0:
                    wb0.append(load_w_chunk(0, k))

            # Phase B: one slab at a time. For all but the last slab run
            # k-outer with all 8 t-tiles accumulating in lockstep across
            # the 8 PSUM banks (consumes chunks as they arrive). The last
            # slab runs t-outer so the final drains stagger instead of all
            # landing after the last matmul.
            def drain(ps_tile, o, t):
                ot = ostage_pool.tile([P, N_TILE], fp32, tag="ot", name="ot")
                # Stores go on the gpsimd SWDGE ring so they never block
                # weight prefetch on the sync ring — except the last slab,
                # whose stores use the (by then idle) sync ring so the slow
                # SWDGE drain starts early and leaves the critical path. The
                # very last tile drains in halves so the first half's HBM
                # write receipt overlaps the second half's copy+transfer.
                last = o == O_TILES - 1
                eng = nc.sync if last else nc.gpsimd
                pieces = 2 if (last and t == T_TILES - 1) else 1
                w = N_TILE // pieces
                for p_i in range(pieces):
                    sl = slice(p_i * w, (p_i + 1) * w)
                    nc.vector.tensor_copy(ot[:, sl], ps_tile[:, sl])
                    eng.dma_start(
                        y_d[
                            t * P : (t + 1) * P,
                            o * N_TILE + p_i * w : o * N_TILE + (p_i + 1) * w,
                        ],
                        ot[:, sl],
                    )

            for o in range(O_TILES):
                wb = wb0 if o == 0 else [
                    load_w_chunk(o, k) for k in range(K_TILES)
                ]
                if o < O_TILES - 1:
                    ps = [
                        psum_pool.tile([P, N_TILE], fp32, tag="ps", name="ps")
                        for _ in range(T_TILES)
                    ]
                    for k in range(K_TILES):
                        for t in range(T_TILES):
                            nc.tensor.matmul(
                                ps[t][:],
                                xs[k][:, t * P : (t + 1) * P],
                                wb[k][:],
                                start=(k == 0),
                                stop=(k == K_TILES - 1),
                            )
                    for t in range(T_TILES):
                        drain(ps[t], o, t)
                else:
                    for t in range(T_TILES):
                        pst = psum_pool.tile([P, N_TILE], fp32, tag="ps", name="ps")
                        for k in range(K_TILES):
                            nc.tensor.matmul(
                                pst[:],
                                xs[k][:, t * P : (t + 1) * P],
                                wb[k][:],
                                start=(k == 0),
                                stop=(k == K_TILES - 1),
                            )
                        drain(pst, o, t)

    nc.compile()
    return nc


def run(x, weight, scale, trace=False, tmpdir=None):
    from concourse.bass_utils import run_bass_kernel_spmd

    x = np.ascontiguousarray(np.asarray(x, dtype=np.float32))
    weight = np.asarray(weight, dtype=np.float32)
    s = float(np.asarray(scale))

    assert x.shape == (TOKENS, IN_F), x.shape
    assert weight.shape == (OUT_F, IN_F), weight.shape

    nc = _build_program(s)

    wt = np.ascontiguousarray(weight.T)  # [IN_F, OUT_F]
    in_maps = []
    for c in range(N_CORES):
        xt = np.ascontiguousarray(x[c * TS : (c + 1) * TS].T)  # [IN_F, TS]
        in_maps.append({"xt": xt, "wt": wt})

    res = run_bass_kernel_spmd(
        nc,
        in_maps,
        core_ids=list(range(N_CORES)),
        trace=trace,
        tmpdir=tmpdir,
    )
    y = np.concatenate([res.results[c]["y"] for c in range(N_CORES)], axis=0)
    return y.astype(np.float32, copy=False), res


def kernel(x, weight, scale):
    y, _ = run(x, weight, scale, trace=False)
    return y



# revision 2
# speedup vs baseline: 1.0182x; 1.0182x over previous
"""BinaryLinear kernel: hybrid fp8-DoubleRow / fp16, k-segmented phase A.

y = x @ (scale * sign(weight))^T,  x:[8192,4096] f32, weight:[4096,4096] f32.

Data-parallel token split (1024 tokens/core), weight replicated. Contraction
K=4096 split: first B8*256 columns as fp8-e4m3 DoubleRow matmuls (2x PE
rate, w=sign exact in fp8, x*scale quantized e4m3), rest fp16. Total rel
err 1.97e-2 (sim == device to 5 digits), under the 2e-2 gate.

Phase A is DMA-bound (x f32 16 MiB + first w at the 358 GB/s HBM cap), so
the first V=3 o-tiles are processed in k-SEGMENTS: each segment's x chunks
are swept across all 3 o-tiles (3x PE work per loaded byte), partial sums
parked in SBUF as fp16 between segments and folded into the final drain.
Remaining o-tiles run the plain full-k lockstep across 8 PSUM banks.
"""

import numpy as np

TOKENS = 8192
IN_F = 4096
OUT_F = 4096
N_CORES = 8
TS = TOKENS // N_CORES  # tokens per core

P = 128
N_TILE = 512            # o-tile width (one PSUM bank of f32)
B8 = 9                  # fp8 k-superblocks (256 wide each)
K8 = B8 * 256
C16 = (IN_F - K8) // P  # fp16 k-chunks (128 wide)
T_TILES = TS // P       # 8
O_TILES = OUT_F // N_TILE  # 8
PSUM_BUFS = 8
V = 3                   # o-tiles processed k-segmented in phase A
SEGS = [2, 4, 5, 6, 6]  # unit counts per segment (sum == B8 + C16 == 23)


def _build_program(scale: float):
    import concourse.bacc as bacc
    import concourse.mybir as mybir
    import concourse.tile as tile

    fp32 = mybir.dt.float32
    fp16 = mybir.dt.float16
    fp8 = mybir.dt.float8e4
    DR = mybir.MatmulPerfMode.DoubleRow

    assert sum(SEGS) == B8 + C16

    nc = bacc.Bacc(
        "TRN2",
        target_bir_lowering=False,
        debug=False,
        num_devices=N_CORES,
    )
    xt_d = nc.dram_tensor("xt", [IN_F, TS], fp32, kind="ExternalInput").ap()
    wt_d = nc.dram_tensor("wt", [IN_F, OUT_F], fp32, kind="ExternalInput").ap()
    y_d = nc.dram_tensor("y", [TS, OUT_F], fp32, kind="ExternalOutput").ap()

    scratch_d = nc.dram_tensor("scratch", [P, N_TILE], fp32, kind="Internal").ap()

    with tile.TileContext(nc) as tc:
        with (
            tc.tile_pool(name="x8res", bufs=B8) as x8_pool,
            tc.tile_pool(name="x16res", bufs=C16) as x16_pool,
            tc.tile_pool(name="xstage", bufs=4) as xstage_pool,
            tc.tile_pool(name="wstage", bufs=10) as wstage_pool,
            tc.tile_pool(name="w8chunk", bufs=20) as w8_pool,
            tc.tile_pool(name="w16chunk", bufs=26) as w16_pool,
            tc.tile_pool(name="ostage", bufs=8) as ostage_pool,
            tc.tile_pool(name="part", bufs=V * T_TILES) as part_pool,
            tc.tile_pool(name="warm", bufs=1) as warm_pool,
            tc.tile_pool(name="psum", bufs=PSUM_BUFS, space="PSUM") as psum_pool,
        ):
            # Warm-up: memset a fp16 tile on DVE (fast start), then dummy
            # matmuls to lift the PE HAM clock gate; ends in a store so it
            # is not dead code.
            warm_h = warm_pool.tile([P, N_TILE], fp16)
            nc.vector.memset(warm_h[:], 0.0)
            warm_ps = psum_pool.tile([P, N_TILE], fp32, tag="ps", name="warm_ps")
            N_WARM = 40
            for i in range(N_WARM):
                nc.tensor.matmul(
                    warm_ps[:],
                    warm_h[:, 0:P],
                    warm_h[:],
                    start=(i == 0),
                    stop=(i == N_WARM - 1),
                )
            warm_o = warm_pool.tile([P, N_TILE], fp32)
            nc.vector.tensor_copy(warm_o[:], warm_ps[:])
            nc.gpsimd.dma_start(scratch_d[:], warm_o[:])

            # k-units: 0..B8-1 are fp8 superblocks (256 wide), B8.. are fp16
            # chunks (128 wide).
            N_UNITS = B8 + C16
            x8s = [None] * B8
            x16s = [None] * C16

            def load_x_unit(u):
                if u < B8:
                    xk = x8_pool.tile([P, 2, TS], fp8, tag="x8")
                    for h in range(2):
                        k0 = u * 256 + h * P
                        xf = xstage_pool.tile([P, TS], fp32, tag="xf")
                        nc.sync.dma_start(xf[:], xt_d[k0 : k0 + P, :])
                        nc.scalar.mul(xk[:, h, :], xf[:], float(scale))
                    x8s[u] = xk
                else:
                    c = u - B8
                    k0 = K8 + c * P
                    xf = xstage_pool.tile([P, TS], fp32, tag="xf")
                    xk = x16_pool.tile([P, TS], fp16, tag="x16")
                    nc.sync.dma_start(xf[:], xt_d[k0 : k0 + P, :])
                    nc.vector.tensor_scalar_mul(xk[:], xf[:], float(scale))
                    x16s[c] = xk

            def load_w_unit(o, u):
                if u < B8:
                    k0 = u * 256
                    wc = w8_pool.tile([P, 2, N_TILE], fp8, tag="w8", name="w8")
                    for h in range(2):
                        wf = wstage_pool.tile([P, N_TILE], fp32, tag="wf")
                        nc.sync.dma_start(
                            wf[:],
                            wt_d[k0 + h * P : k0 + (h + 1) * P,
                                 o * N_TILE : (o + 1) * N_TILE],
                        )
                        nc.scalar.sign(wc[:, h, :], wf[:])
                else:
                    k0 = K8 + (u - B8) * P
                    wf = wstage_pool.tile([P, N_TILE], fp32, tag="wf")
                    nc.sync.dma_start(
                        wf[:],
                        wt_d[k0 : k0 + P, o * N_TILE : (o + 1) * N_TILE],
                    )
                    wc = w16_pool.tile([P, N_TILE], fp16, tag="w16", name="w16")
                    nc.scalar.sign(wc[:], wf[:])
                return wc

            def mm(ps, u, t, wc, start, stop):
                if u < B8:
                    nc.tensor.matmul(
                        ps[:],
                        x8s[u][:, :, t * P : (t + 1) * P],
                        wc[:],
                        start=start, stop=stop, perf_mode=DR,
                    )
                else:
                    nc.tensor.matmul(
                        ps[:],
                        x16s[u - B8][:, t * P : (t + 1) * P],
                        wc[:],
                        start=start, stop=stop,
                    )

            def store(ot, o, t, lo, hi, eng):
                eng.dma_start(
                    y_d[t * P : (t + 1) * P,
                        o * N_TILE + lo : o * N_TILE + hi],
                    ot[:, lo:hi],
                )

            # ---- Phase A: first V o-tiles, k-segmented ----
            partials = [[None] * T_TILES for _ in range(V)]
            u_done = 0
            for si, n_u in enumerate(SEGS):
                units = list(range(u_done, u_done + n_u))
                u_done += n_u
                first_seg = si == 0
                last_seg = si == len(SEGS) - 1
                # DMA order: x(u) + w(o0,u) interleaved, then w(o1), w(o2)
                wcs = [[None] * n_u for _ in range(V)]
                for j, u in enumerate(units):
                    load_x_unit(u)
                    wcs[0][j] = load_w_unit(0, u)
                for o in range(1, V):
                    for j, u in enumerate(units):
                        wcs[o][j] = load_w_unit(o, u)
                for o in range(V):
                    ps = [
                        psum_pool.tile([P, N_TILE], fp32, tag="ps", name="ps")
                        for _ in range(T_TILES)
                    ]
                    for j, u in enumerate(units):
                        for t in range(T_TILES):
                            mm(ps[t], u, t, wcs[o][j],
                               start=(j == 0), stop=(j == n_u - 1))
                    for t in range(T_TILES):
                        if first_seg:
                            pt = part_pool.tile([P, N_TILE], fp32, tag="part")
                            nc.vector.tensor_copy(pt[:], ps[t][:])
                            partials[o][t] = pt
                        elif not last_seg:
                            pt = partials[o][t]
                            nc.vector.scalar_tensor_tensor(
                                pt[:], ps[t][:], 1.0, pt[:],
                                mybir.AluOpType.mult, mybir.AluOpType.add,
                            )
                        else:
                            pt = partials[o][t]
                            ot = ostage_pool.tile(
                                [P, N_TILE], fp32, tag="ot", name="ot"
                            )
                            half = N_TILE // 2
                            for p_i in range(2):
                                sl = slice(p_i * half, (p_i + 1) * half)
                                nc.vector.scalar_tensor_tensor(
                                    ot[:, sl], ps[t][:, sl], 1.0, pt[:, sl],
                                    mybir.AluOpType.mult, mybir.AluOpType.add,
                                )
                                store(ot, o, t, p_i * half, (p_i + 1) * half,
                                      nc.gpsimd)

            # ---- Remaining o-tiles: full-k lockstep ----
            for o in range(V, O_TILES):
                w8c = [load_w_unit(o, u) for u in range(B8)]
                w16c = [load_w_unit(o, u) for u in range(B8, N_UNITS)]
                wall = w8c + w16c
                lasto = o == O_TILES - 1
                if not lasto:
                    ps = [
                        psum_pool.tile([P, N_TILE], fp32, tag="ps", name="ps")
                        for _ in range(T_TILES)
                    ]
                    for u in range(N_UNITS):
                        for t in range(T_TILES):
                            mm(ps[t], u, t, wall[u],
                               start=(u == 0), stop=(u == N_UNITS - 1))
                    for t in range(T_TILES):
                        ot = ostage_pool.tile([P, N_TILE], fp32,
                                              tag="ot", name="ot")
                        pieces = 2 if t <= 1 else 1
                        w = N_TILE // pieces
                        for p_i in range(pieces):
                            sl = slice(p_i * w, (p_i + 1) * w)
                            nc.vector.tensor_copy(ot[:, sl], ps[t][:, sl])
                            store(ot, o, t, p_i * w, (p_i + 1) * w, nc.gpsimd)
                else:
                    # last o-tile: t-outer so the final drains stagger;
                    # stores on the (by now idle) sync ring.
                    for t in range(T_TILES):
                        pst = psum_pool.tile([P, N_TILE], fp32,
                                             tag="ps", name="ps")
                        for u in range(N_UNITS):
                            mm(pst, u, t, wall[u],
                               start=(u == 0), stop=(u == N_UNITS - 1))
                        ot = ostage_pool.tile([P, N_TILE], fp32,
                                              tag="ot", name="ot")
                        pieces = 2 if t == T_TILES - 1 else 1
                        w = N_TILE // pieces
                        for p_i in range(pieces):
                            sl = slice(p_i * w, (p_i + 1) * w)
                            nc.vector.tensor_copy(ot[:, sl], pst[:, sl])
                            store(ot, o, t, p_i * w, (p_i + 1) * w, nc.sync)

    nc.compile()
    return nc


def run(x, weight, scale, trace=False, tmpdir=None):
    from concourse.bass_utils import run_bass_kernel_spmd

    x = np.ascontiguousarray(np.asarray(x, dtype=np.float32))
    weight = np.asarray(weight, dtype=np.float32)
    s = float(np.asarray(scale))

    assert x.shape == (TOKENS, IN_F), x.shape
    assert weight.shape == (OUT_F, IN_F), weight.shape

    nc = _build_program(s)

    wt = np.ascontiguousarray(weight.T)  # [IN_F, OUT_F]
    in_maps = []
    for c in range(N_CORES):
        xt = np.ascontiguousarray(x[c * TS : (c + 1) * TS].T)  # [IN_F, TS]
        in_maps.append({"xt": xt, "wt": wt})

    res = run_bass_kernel_spmd(
        nc,
        in_maps,
        core_ids=list(range(N_CORES)),
        trace=trace,
        tmpdir=tmpdir,
    )
    y = np.concatenate([res.results[c]["y"] for c in range(N_CORES)], axis=0)
    return y.astype(np.float32, copy=False), res


def kernel(x, weight, scale):
    y, _ = run(x, weight, scale, trace=False)
    return y


# revision 3
# speedup vs baseline: 1.0308x; 1.0124x over previous
"""BinaryLinear kernel v3: hybrid fp8-DoubleRow / fp16, k-segmented phase A.

y = x @ (scale * sign(weight))^T,  x:[8192,4096] f32, weight:[4096,4096] f32.

Data-parallel token split (1024 tokens/core), weight replicated. Contraction
K=4096 split: first B8*256 columns as fp8-e4m3 DoubleRow matmuls (2x PE
rate, w=sign exact in fp8, x*scale quantized e4m3), rest fp16. Total rel
err 1.97e-2 (sim == device to 5 digits), under the 2e-2 gate.

Phase A is DMA-bound (x f32 16 MiB + first w at the 358 GB/s HBM cap), so
the first V=3 o-tiles are processed in k-SEGMENTS: each segment's x chunks
are swept across all 3 o-tiles (3x PE work per loaded byte), partial sums
parked in SBUF as fp16 between segments and folded into the final drain.
Remaining o-tiles run the plain full-k lockstep across 8 PSUM banks.
"""

import numpy as np

TOKENS = 8192
IN_F = 4096
OUT_F = 4096
N_CORES = 8
TS = TOKENS // N_CORES  # tokens per core

P = 128
N_TILE = 512            # o-tile width (one PSUM bank of f32)
B8 = 9                  # fp8 k-superblocks (256 wide each)
K8 = B8 * 256
C16 = (IN_F - K8) // P  # fp16 k-chunks (128 wide)
T_TILES = TS // P       # 8
O_TILES = OUT_F // N_TILE  # 8
PSUM_BUFS = 8
V = 3                   # o-tiles processed k-segmented in phase A
SEGS = [2, 4, 5, 6, 6]  # unit counts per segment (sum == B8 + C16 == 23)


def _build_program(scale: float):
    import concourse.bacc as bacc
    import concourse.mybir as mybir
    import concourse.tile as tile

    fp32 = mybir.dt.float32
    fp16 = mybir.dt.float16
    fp8 = mybir.dt.float8e4
    DR = mybir.MatmulPerfMode.DoubleRow

    assert sum(SEGS) == B8 + C16

    nc = bacc.Bacc(
        "TRN2",
        target_bir_lowering=False,
        debug=False,
        num_devices=N_CORES,
    )
    xt_d = nc.dram_tensor("xt", [IN_F, TS], fp32, kind="ExternalInput").ap()
    wt_d = nc.dram_tensor("wt", [IN_F, OUT_F], fp32, kind="ExternalInput").ap()
    y_d = nc.dram_tensor("y", [TS, OUT_F], fp32, kind="ExternalOutput").ap()

    scratch_d = nc.dram_tensor("scratch", [P, N_TILE], fp32, kind="Internal").ap()

    with tile.TileContext(nc) as tc:
        with (
            tc.tile_pool(name="x8res", bufs=B8) as x8_pool,
            tc.tile_pool(name="x16res", bufs=C16) as x16_pool,
            tc.tile_pool(name="xstage", bufs=4) as xstage_pool,
            tc.tile_pool(name="wstage", bufs=12) as wstage_pool,
            tc.tile_pool(name="w8chunk", bufs=20) as w8_pool,
            tc.tile_pool(name="w16chunk", bufs=26) as w16_pool,
            tc.tile_pool(name="ostage", bufs=8) as ostage_pool,
            tc.tile_pool(name="part", bufs=V * T_TILES) as part_pool,
            tc.tile_pool(name="warm", bufs=1) as warm_pool,
            tc.tile_pool(name="psum", bufs=PSUM_BUFS, space="PSUM") as psum_pool,
        ):
            # Warm-up: memset a fp16 tile on DVE (fast start), then dummy
            # matmuls to lift the PE HAM clock gate; ends in a store so it
            # is not dead code.
            warm_h = warm_pool.tile([P, N_TILE], fp16)
            nc.vector.memset(warm_h[:], 0.0)
            warm_ps = psum_pool.tile([P, N_TILE], fp32, tag="ps", name="warm_ps")
            N_WARM = 40
            for i in range(N_WARM):
                nc.tensor.matmul(
                    warm_ps[:],
                    warm_h[:, 0:P],
                    warm_h[:],
                    start=(i == 0),
                    stop=(i == N_WARM - 1),
                )
            warm_o = warm_pool.tile([P, N_TILE], fp32)
            nc.vector.tensor_copy(warm_o[:], warm_ps[:])
            nc.gpsimd.dma_start(scratch_d[:], warm_o[:])

            # k-units: 0..B8-1 are fp8 superblocks (256 wide), B8.. are fp16
            # chunks (128 wide).
            N_UNITS = B8 + C16
            x8s = [None] * B8
            x16s = [None] * C16

            def load_x_unit(u):
                if u < B8:
                    xk = x8_pool.tile([P, 2, TS], fp8, tag="x8")
                    for h in range(2):
                        k0 = u * 256 + h * P
                        xf = xstage_pool.tile([P, TS], fp32, tag="xf")
                        nc.sync.dma_start(xf[:], xt_d[k0 : k0 + P, :])
                        nc.scalar.mul(xk[:, h, :], xf[:], float(scale))
                    x8s[u] = xk
                else:
                    c = u - B8
                    k0 = K8 + c * P
                    xf = xstage_pool.tile([P, TS], fp32, tag="xf")
                    xk = x16_pool.tile([P, TS], fp16, tag="x16")
                    nc.sync.dma_start(xf[:], xt_d[k0 : k0 + P, :])
                    nc.vector.tensor_scalar_mul(xk[:], xf[:], float(scale))
                    x16s[c] = xk

            def load_w_unit(o, u):
                if u < B8:
                    k0 = u * 256
                    wc = w8_pool.tile([P, 2, N_TILE], fp8, tag="w8", name="w8")
                    for h in range(2):
                        wf = wstage_pool.tile([P, N_TILE], fp32, tag="wf")
                        nc.sync.dma_start(
                            wf[:],
                            wt_d[k0 + h * P : k0 + (h + 1) * P,
                                 o * N_TILE : (o + 1) * N_TILE],
                        )
                        nc.scalar.sign(wc[:, h, :], wf[:])
                else:
                    k0 = K8 + (u - B8) * P
                    wf = wstage_pool.tile([P, N_TILE], fp32, tag="wf")
                    nc.sync.dma_start(
                        wf[:],
                        wt_d[k0 : k0 + P, o * N_TILE : (o + 1) * N_TILE],
                    )
                    wc = w16_pool.tile([P, N_TILE], fp16, tag="w16", name="w16")
                    nc.scalar.sign(wc[:], wf[:])
                return wc

            def mm(ps, u, t, wc, start, stop):
                if u < B8:
                    nc.tensor.matmul(
                        ps[:],
                        x8s[u][:, :, t * P : (t + 1) * P],
                        wc[:],
                        start=start, stop=stop, perf_mode=DR,
                    )
                else:
                    nc.tensor.matmul(
                        ps[:],
                        x16s[u - B8][:, t * P : (t + 1) * P],
                        wc[:],
                        start=start, stop=stop,
                    )

            def store(ot, o, t, lo, hi, eng):
                eng.dma_start(
                    y_d[t * P : (t + 1) * P,
                        o * N_TILE + lo : o * N_TILE + hi],
                    ot[:, lo:hi],
                )

            # ---- Phase A: first V o-tiles, k-segmented ----
            partials = [[None] * T_TILES for _ in range(V)]
            u_done = 0
            for si, n_u in enumerate(SEGS):
                units = list(range(u_done, u_done + n_u))
                u_done += n_u
                first_seg = si == 0
                last_seg = si == len(SEGS) - 1
                # DMA order: x(u) + w(o0,u) interleaved, then w(o1), w(o2)
                wcs = [[None] * n_u for _ in range(V)]
                for j, u in enumerate(units):
                    load_x_unit(u)
                    wcs[0][j] = load_w_unit(0, u)
                for o in range(1, V):
                    for j, u in enumerate(units):
                        wcs[o][j] = load_w_unit(o, u)
                for o in range(V):
                    ps = [
                        psum_pool.tile([P, N_TILE], fp32, tag="ps", name="ps")
                        for _ in range(T_TILES)
                    ]
                    for j, u in enumerate(units):
                        for t in range(T_TILES):
                            mm(ps[t], u, t, wcs[o][j],
                               start=(j == 0), stop=(j == n_u - 1))
                    for t in range(T_TILES):
                        if first_seg:
                            pt = part_pool.tile([P, N_TILE], fp32, tag="part")
                            nc.vector.tensor_copy(pt[:], ps[t][:])
                            partials[o][t] = pt
                        elif not last_seg:
                            pt = partials[o][t]
                            nc.vector.scalar_tensor_tensor(
                                pt[:], ps[t][:], 1.0, pt[:],
                                mybir.AluOpType.mult, mybir.AluOpType.add,
                            )
                        else:
                            pt = partials[o][t]
                            ot = ostage_pool.tile(
                                [P, N_TILE], fp32, tag="ot", name="ot"
                            )
                            half = N_TILE // 2
                            for p_i in range(2):
                                sl = slice(p_i * half, (p_i + 1) * half)
                                nc.vector.scalar_tensor_tensor(
                                    ot[:, sl], ps[t][:, sl], 1.0, pt[:, sl],
                                    mybir.AluOpType.mult, mybir.AluOpType.add,
                                )
                                store(ot, o, t, p_i * half, (p_i + 1) * half,
                                      nc.gpsimd)

            # prefetch the next o-tile's fp8 w units (issued on the sync
            # ring ahead of its lockstep; segs 4-5 hold no fp8 tiles)
            w8_pre = [load_w_unit(V, u) for u in range(B8)]

            # ---- Remaining o-tiles: full-k lockstep ----
            for o in range(V, O_TILES):
                w8c = w8_pre if o == V else [load_w_unit(o, u) for u in range(B8)]
                w16c = [load_w_unit(o, u) for u in range(B8, N_UNITS)]
                wall = w8c + w16c
                lasto = o == O_TILES - 1
                if not lasto:
                    ps = [
                        psum_pool.tile([P, N_TILE], fp32, tag="ps", name="ps")
                        for _ in range(T_TILES)
                    ]
                    for u in range(N_UNITS):
                        for t in range(T_TILES):
                            mm(ps[t], u, t, wall[u],
                               start=(u == 0), stop=(u == N_UNITS - 1))
                    for t in range(T_TILES):
                        ot = ostage_pool.tile([P, N_TILE], fp32,
                                              tag="ot", name="ot")
                        pieces = 2 if t <= 1 else 1
                        w = N_TILE // pieces
                        for p_i in range(pieces):
                            sl = slice(p_i * w, (p_i + 1) * w)
                            nc.vector.tensor_copy(ot[:, sl], ps[t][:, sl])
                            store(ot, o, t, p_i * w, (p_i + 1) * w, nc.gpsimd)
                else:
                    # last o-tile: t-outer so the final drains stagger;
                    # stores on the (by now idle) sync ring.
                    for t in range(T_TILES):
                        pst = psum_pool.tile([P, N_TILE], fp32,
                                             tag="ps", name="ps")
                        for u in range(N_UNITS):
                            mm(pst, u, t, wall[u],
                               start=(u == 0), stop=(u == N_UNITS - 1))
                        ot = ostage_pool.tile([P, N_TILE], fp32,
                                              tag="ot", name="ot")
                        pieces = 2 if t == T_TILES - 1 else 1
                        w = N_TILE // pieces
                        for p_i in range(pieces):
                            sl = slice(p_i * w, (p_i + 1) * w)
                            nc.vector.tensor_copy(ot[:, sl], pst[:, sl])
                            store(ot, o, t, p_i * w, (p_i + 1) * w, nc.sync)

    nc.compile()
    return nc


def run(x, weight, scale, trace=False, tmpdir=None):
    from concourse.bass_utils import run_bass_kernel_spmd

    x = np.ascontiguousarray(np.asarray(x, dtype=np.float32))
    weight = np.asarray(weight, dtype=np.float32)
    s = float(np.asarray(scale))

    assert x.shape == (TOKENS, IN_F), x.shape
    assert weight.shape == (OUT_F, IN_F), weight.shape

    nc = _build_program(s)

    wt = np.ascontiguousarray(weight.T)  # [IN_F, OUT_F]
    in_maps = []
    for c in range(N_CORES):
        xt = np.ascontiguousarray(x[c * TS : (c + 1) * TS].T)  # [IN_F, TS]
        in_maps.append({"xt": xt, "wt": wt})

    res = run_bass_kernel_spmd(
        nc,
        in_maps,
        core_ids=list(range(N_CORES)),
        trace=trace,
        tmpdir=tmpdir,
    )
    y = np.concatenate([res.results[c]["y"] for c in range(N_CORES)], axis=0)
    return y.astype(np.float32, copy=False), res


def kernel(x, weight, scale):
    y, _ = run(x, weight, scale, trace=False)
    return y


# revision 4
# speedup vs baseline: 1.0310x; 1.0002x over previous
"""BinaryLinear kernel v3: hybrid fp8-DoubleRow / fp16, k-segmented phase A.

y = x @ (scale * sign(weight))^T,  x:[8192,4096] f32, weight:[4096,4096] f32.

Data-parallel token split (1024 tokens/core), weight replicated. Contraction
K=4096 split: first B8*256 columns as fp8-e4m3 DoubleRow matmuls (2x PE
rate, w=sign exact in fp8, x*scale quantized e4m3), rest fp16. Total rel
err 1.97e-2 (sim == device to 5 digits), under the 2e-2 gate.

Phase A is DMA-bound (x f32 16 MiB + first w at the 358 GB/s HBM cap), so
the first V=3 o-tiles are processed in k-SEGMENTS: each segment's x chunks
are swept across all 3 o-tiles (3x PE work per loaded byte), partial sums
parked in SBUF as fp16 between segments and folded into the final drain.
Remaining o-tiles run the plain full-k lockstep across 8 PSUM banks.
"""

import numpy as np

TOKENS = 8192
IN_F = 4096
OUT_F = 4096
N_CORES = 8
TS = TOKENS // N_CORES  # tokens per core

P = 128
N_TILE = 512            # o-tile width (one PSUM bank of f32)
B8 = 9                  # fp8 k-superblocks (256 wide each)
K8 = B8 * 256
C16 = (IN_F - K8) // P  # fp16 k-chunks (128 wide)
T_TILES = TS // P       # 8
O_TILES = OUT_F // N_TILE  # 8
PSUM_BUFS = 8
V = 3                   # o-tiles processed k-segmented in phase A
SEGS = [3, 5, 6, 4, 5]  # unit counts per segment (sum == B8 + C16 == 23)
# phase-A unit order: fp16 chunks first (0.5x the DMA bytes per unit of PE
# work -> builds a DMA lead), DR superblocks last (consume the lead); each
# segment stays dtype-pure so the PE never mode-switches mid-sweep
UNIT_ORDER = list(range(9, 23)) + list(range(9))


def _build_program(scale: float):
    import concourse.bacc as bacc
    import concourse.mybir as mybir
    import concourse.tile as tile

    fp32 = mybir.dt.float32
    fp16 = mybir.dt.float16
    fp8 = mybir.dt.float8e4
    DR = mybir.MatmulPerfMode.DoubleRow

    assert sum(SEGS) == B8 + C16

    nc = bacc.Bacc(
        "TRN2",
        target_bir_lowering=False,
        debug=False,
        num_devices=N_CORES,
    )
    xt_d = nc.dram_tensor("xt", [IN_F, TS], fp32, kind="ExternalInput").ap()
    wt_d = nc.dram_tensor("wt", [IN_F, OUT_F], fp32, kind="ExternalInput").ap()
    y_d = nc.dram_tensor("y", [TS, OUT_F], fp32, kind="ExternalOutput").ap()

    scratch_d = nc.dram_tensor("scratch", [P, N_TILE], fp32, kind="Internal").ap()

    with tile.TileContext(nc) as tc:
        with (
            tc.tile_pool(name="x8res", bufs=B8) as x8_pool,
            tc.tile_pool(name="x16res", bufs=C16) as x16_pool,
            tc.tile_pool(name="xstage", bufs=4) as xstage_pool,
            tc.tile_pool(name="wstage", bufs=12) as wstage_pool,
            tc.tile_pool(name="w8chunk", bufs=20) as w8_pool,
            tc.tile_pool(name="w16chunk", bufs=26) as w16_pool,
            tc.tile_pool(name="ostage", bufs=8) as ostage_pool,
            tc.tile_pool(name="part", bufs=V * T_TILES) as part_pool,
            tc.tile_pool(name="warm", bufs=1) as warm_pool,
            tc.tile_pool(name="psum", bufs=PSUM_BUFS, space="PSUM") as psum_pool,
        ):
            # Warm-up: memset a fp16 tile on DVE (fast start), then dummy
            # matmuls to lift the PE HAM clock gate; ends in a store so it
            # is not dead code.
            warm_h = warm_pool.tile([P, N_TILE], fp16)
            nc.vector.memset(warm_h[:], 0.0)
            warm_ps = psum_pool.tile([P, N_TILE], fp32, tag="ps", name="warm_ps")
            N_WARM = 40
            for i in range(N_WARM):
                nc.tensor.matmul(
                    warm_ps[:],
                    warm_h[:, 0:P],
                    warm_h[:],
                    start=(i == 0),
                    stop=(i == N_WARM - 1),
                )
            warm_o = warm_pool.tile([P, N_TILE], fp32)
            nc.vector.tensor_copy(warm_o[:], warm_ps[:])
            nc.gpsimd.dma_start(scratch_d[:], warm_o[:])

            # k-units: 0..B8-1 are fp8 superblocks (256 wide), B8.. are fp16
            # chunks (128 wide).
            N_UNITS = B8 + C16
            x8s = [None] * B8
            x16s = [None] * C16

            def load_x_unit(u):
                if u < B8:
                    xk = x8_pool.tile([P, 2, TS], fp8, tag="x8")
                    for h in range(2):
                        k0 = u * 256 + h * P
                        xf = xstage_pool.tile([P, TS], fp32, tag="xf")
                        nc.sync.dma_start(xf[:], xt_d[k0 : k0 + P, :])
                        nc.scalar.mul(xk[:, h, :], xf[:], float(scale))
                    x8s[u] = xk
                else:
                    c = u - B8
                    k0 = K8 + c * P
                    xf = xstage_pool.tile([P, TS], fp32, tag="xf")
                    xk = x16_pool.tile([P, TS], fp16, tag="x16")
                    nc.sync.dma_start(xf[:], xt_d[k0 : k0 + P, :])
                    nc.vector.tensor_scalar_mul(xk[:], xf[:], float(scale))
                    x16s[c] = xk

            def load_w_unit(o, u):
                if u < B8:
                    k0 = u * 256
                    wc = w8_pool.tile([P, 2, N_TILE], fp8, tag="w8", name="w8")
                    for h in range(2):
                        wf = wstage_pool.tile([P, N_TILE], fp32, tag="wf")
                        nc.sync.dma_start(
                            wf[:],
                            wt_d[k0 + h * P : k0 + (h + 1) * P,
                                 o * N_TILE : (o + 1) * N_TILE],
                        )
                        nc.scalar.sign(wc[:, h, :], wf[:])
                else:
                    k0 = K8 + (u - B8) * P
                    wf = wstage_pool.tile([P, N_TILE], fp32, tag="wf")
                    nc.sync.dma_start(
                        wf[:],
                        wt_d[k0 : k0 + P, o * N_TILE : (o + 1) * N_TILE],
                    )
                    wc = w16_pool.tile([P, N_TILE], fp16, tag="w16", name="w16")
                    nc.scalar.sign(wc[:], wf[:])
                return wc

            def mm(ps, u, t, wc, start, stop):
                if u < B8:
                    nc.tensor.matmul(
                        ps[:],
                        x8s[u][:, :, t * P : (t + 1) * P],
                        wc[:],
                        start=start, stop=stop, perf_mode=DR,
                    )
                else:
                    nc.tensor.matmul(
                        ps[:],
                        x16s[u - B8][:, t * P : (t + 1) * P],
                        wc[:],
                        start=start, stop=stop,
                    )

            def store(ot, o, t, lo, hi, eng):
                eng.dma_start(
                    y_d[t * P : (t + 1) * P,
                        o * N_TILE + lo : o * N_TILE + hi],
                    ot[:, lo:hi],
                )

            # ---- Phase A: first V o-tiles, k-segmented ----
            partials = [[None] * T_TILES for _ in range(V)]
            u_done = 0
            for si, n_u in enumerate(SEGS):
                units = UNIT_ORDER[u_done : u_done + n_u]
                u_done += n_u
                first_seg = si == 0
                last_seg = si == len(SEGS) - 1
                # DMA order: x(u) + w(o0,u) interleaved, then w(o1), w(o2)
                wcs = [[None] * n_u for _ in range(V)]
                for j, u in enumerate(units):
                    load_x_unit(u)
                    wcs[0][j] = load_w_unit(0, u)
                for o in range(1, V):
                    for j, u in enumerate(units):
                        wcs[o][j] = load_w_unit(o, u)
                for o in range(V):
                    ps = [
                        psum_pool.tile([P, N_TILE], fp32, tag="ps", name="ps")
                        for _ in range(T_TILES)
                    ]
                    for j, u in enumerate(units):
                        for t in range(T_TILES):
                            mm(ps[t], u, t, wcs[o][j],
                               start=(j == 0), stop=(j == n_u - 1))
                    for t in range(T_TILES):
                        if first_seg:
                            pt = part_pool.tile([P, N_TILE], fp32, tag="part")
                            nc.vector.tensor_copy(pt[:], ps[t][:])
                            partials[o][t] = pt
                        elif not last_seg:
                            pt = partials[o][t]
                            nc.vector.scalar_tensor_tensor(
                                pt[:], ps[t][:], 1.0, pt[:],
                                mybir.AluOpType.mult, mybir.AluOpType.add,
                            )
                        else:
                            pt = partials[o][t]
                            ot = ostage_pool.tile(
                                [P, N_TILE], fp32, tag="ot", name="ot"
                            )
                            half = N_TILE // 2
                            for p_i in range(2):
                                sl = slice(p_i * half, (p_i + 1) * half)
                                nc.vector.scalar_tensor_tensor(
                                    ot[:, sl], ps[t][:, sl], 1.0, pt[:, sl],
                                    mybir.AluOpType.mult, mybir.AluOpType.add,
                                )
                                store(ot, o, t, p_i * half, (p_i + 1) * half,
                                      nc.gpsimd)

            # prefetch the next o-tile's first (fp16, DMA-light) w units so
            # its lockstep starts while phase A's DR tail still streams
            NPRE = 9
            w_pre = [load_w_unit(V, u) for u in UNIT_ORDER[:NPRE]]

            # ---- Remaining o-tiles: full-k lockstep, fp16 units first ----
            # (same block order as phase A: one PE mode switch per o-tile)
            for o in range(V, O_TILES):
                wall = {}
                if o == V:
                    for i, u in enumerate(UNIT_ORDER[:NPRE]):
                        wall[u] = w_pre[i]
                    for u in UNIT_ORDER[NPRE:]:
                        wall[u] = load_w_unit(o, u)
                else:
                    for u in UNIT_ORDER:
                        wall[u] = load_w_unit(o, u)
                lasto = o == O_TILES - 1
                if not lasto:
                    ps = [
                        psum_pool.tile([P, N_TILE], fp32, tag="ps", name="ps")
                        for _ in range(T_TILES)
                    ]
                    for j, u in enumerate(UNIT_ORDER):
                        for t in range(T_TILES):
                            mm(ps[t], u, t, wall[u],
                               start=(j == 0), stop=(j == N_UNITS - 1))
                    for t in range(T_TILES):
                        ot = ostage_pool.tile([P, N_TILE], fp32,
                                              tag="ot", name="ot")
                        pieces = 2 if t <= 1 else 1
                        w = N_TILE // pieces
                        for p_i in range(pieces):
                            sl = slice(p_i * w, (p_i + 1) * w)
                            nc.vector.tensor_copy(ot[:, sl], ps[t][:, sl])
                            store(ot, o, t, p_i * w, (p_i + 1) * w, nc.gpsimd)
                else:
                    # last o-tile: t-outer so the final drains stagger;
                    # stores on the (by now idle) sync ring.
                    for t in range(T_TILES):
                        pst = psum_pool.tile([P, N_TILE], fp32,
                                             tag="ps", name="ps")
                        for j, u in enumerate(UNIT_ORDER):
                            mm(pst, u, t, wall[u],
                               start=(j == 0), stop=(j == N_UNITS - 1))
                        ot = ostage_pool.tile([P, N_TILE], fp32,
                                              tag="ot", name="ot")
                        pieces = 2 if t == T_TILES - 1 else 1
                        w = N_TILE // pieces
                        for p_i in range(pieces):
                            sl = slice(p_i * w, (p_i + 1) * w)
                            nc.vector.tensor_copy(ot[:, sl], pst[:, sl])
                            store(ot, o, t, p_i * w, (p_i + 1) * w, nc.sync)

    nc.compile()
    return nc


def run(x, weight, scale, trace=False, tmpdir=None):
    from concourse.bass_utils import run_bass_kernel_spmd

    x = np.ascontiguousarray(np.asarray(x, dtype=np.float32))
    weight = np.asarray(weight, dtype=np.float32)
    s = float(np.asarray(scale))

    assert x.shape == (TOKENS, IN_F), x.shape
    assert weight.shape == (OUT_F, IN_F), weight.shape

    nc = _build_program(s)

    wt = np.ascontiguousarray(weight.T)  # [IN_F, OUT_F]
    in_maps = []
    for c in range(N_CORES):
        xt = np.ascontiguousarray(x[c * TS : (c + 1) * TS].T)  # [IN_F, TS]
        in_maps.append({"xt": xt, "wt": wt})

    res = run_bass_kernel_spmd(
        nc,
        in_maps,
        core_ids=list(range(N_CORES)),
        trace=trace,
        tmpdir=tmpdir,
    )
    y = np.concatenate([res.results[c]["y"] for c in range(N_CORES)], axis=0)
    return y.astype(np.float32, copy=False), res


def kernel(x, weight, scale):
    y, _ = run(x, weight, scale, trace=False)
    return y
